# revision 1
# baseline (speedup 1.0000x reference)
"""HGNN layer on 8 Trainium2 NeuronCores (Bass/Tile).

Reference computation:
    x1 = x @ W1                                    [N, F]
    w = softmax(where(seq > 0, 1, -9e15))          uniform over valid slots
    edge = relu(sum_l w[e,l] * x1[seq[e,l]])       [E, F]
    e1 = edge @ W2                                 [E, F]
    uw = softmax(where(useq > 0, 1, -9e15))
    node = sum_l uw[n,l] * e1[useq[n,l]]           [N, F]

Strategy (8-way SPMD):
  - shard nodes/edges by rows; W1/W2 replicated
  - x1 shard computed on each core -> AllGather to a full x1 table (+zero row)
  - stage 1: batched indirect-DMA row gathers (128 rows/instr) from x1 table,
    in-place halving-tree reduce, uniform-weight correction via per-row count,
    relu, @W2 (PE transpose + matmul) -> e1 shard -> AllGather (+zero row)
  - stage 2: same gather+reduce from e1 table -> node shard -> output
  - padding slots (id 0) are remapped host-side to the zero row; counts are
    computed on device; the all-padding case adds row0 of the table exactly
    as the reference softmax does.
"""

import sys

sys.path.insert(0, "/opt/trn_rl_repo")

import numpy as np

N = 50000
E = 25000
F = 256
L = 32
P = 128
NC_COUNT = 8
NSH = N // NC_COUNT        # 6250 nodes per core
ESH = E // NC_COUNT        # 3125 edges per core
NSH_PAD = 6272             # 49 tiles
ESH_PAD = 3200             # 25 tiles
N_TILES_NODE = NSH_PAD // P
N_TILES_EDGE = ESH_PAD // P


def build_program():
    from concourse import bacc, bass, mybir, tile  # noqa: F401
    from concourse.masks import make_identity

    fp32 = mybir.dt.float32
    i32 = mybir.dt.int32

    nc = bacc.Bacc("TRN2", target_bir_lowering=False, debug=False,
                   num_devices=NC_COUNT)

    xts = nc.dram_tensor("xts", [F, NSH_PAD], fp32, kind="ExternalInput").ap()
    w1 = nc.dram_tensor("w1", [F, F], fp32, kind="ExternalInput").ap()
    w2 = nc.dram_tensor("w2", [F, F], fp32, kind="ExternalInput").ap()
    seqp = nc.dram_tensor("seqp", [ESH_PAD, L], i32, kind="ExternalInput").ap()
    useqp = nc.dram_tensor("useqp", [NSH_PAD, L], i32, kind="ExternalInput").ap()
    out = nc.dram_tensor("out", [NSH_PAD, F], fp32, kind="ExternalOutput").ap()

    AL = mybir.AluOpType

    with tile.TileContext(nc) as tc:
        with (
            tc.tile_pool(name="cst", bufs=1) as cst,
            tc.tile_pool(name="lhs", bufs=3) as lhsp,
            tc.tile_pool(name="sb", bufs=3) as sbp,
            tc.tile_pool(name="idx", bufs=8) as idxp,
            tc.tile_pool(name="gb", bufs=3) as gbp,
            tc.tile_pool(name="st", bufs=4) as stp,
            tc.tile_pool(name="ps", bufs=2, space="PSUM") as psp,
            tc.tile_pool(name="pst", bufs=2, space="PSUM") as pstp,
            tc.tile_pool(name="dram", bufs=1, space="DRAM") as dram,
        ):
            # ---------- constants ----------
            ident = cst.tile([P, P], fp32)
            make_identity(nc, ident[:])
            w1sb = [cst.tile([P, F], fp32, name=f"w1k{k}") for k in range(2)]
            w2sb = [cst.tile([P, F], fp32, name=f"w2k{k}") for k in range(2)]
            for k in range(2):
                nc.sync.dma_start(out=w1sb[k][:], in_=w1[k * P:(k + 1) * P, :])
                nc.sync.dma_start(out=w2sb[k][:], in_=w2[k * P:(k + 1) * P, :])
            zrow = cst.tile([1, F], fp32)
            nc.vector.memset(zrow[:], 0.0)
            ones1p = cst.tile([1, P], fp32)
            nc.vector.memset(ones1p[:], 1.0)

            # ---------- DRAM scratch ----------
            # each core's shard carries a trailing zero row so the AllGather
            # output (the gather table) contains zero rows without a second
            # writer on the Shared tensor
            x1loc = dram.tile([NSH + 1, F], fp32)
            x1tab = dram.tile([NC_COUNT * (NSH + 1), F], fp32, addr_space="Shared")
            e1loc = dram.tile([ESH + 1, F], fp32)
            e1tab = dram.tile([NC_COUNT * (ESH + 1), F], fp32, addr_space="Shared")

            # ---------- stage 0: x1 shard = x_shard @ W1 ----------
            with nc.named_scope("stage0"):
                for t in range(N_TILES_NODE):
                    pr = min(P, NSH - t * P)  # rows to store
                    ps0 = psp.tile([P, F], fp32, tag="mm")
                    for kc in range(2):
                        lt = lhsp.tile([P, P], fp32, tag="lhs")
                        nc.sync.dma_start(
                            out=lt[:],
                            in_=xts[kc * P:(kc + 1) * P, t * P:(t + 1) * P],
                        )
                        nc.tensor.matmul(ps0[:], lt[:], w1sb[kc][:],
                                         start=(kc == 0), stop=(kc == 1))
                    if pr > 0:
                        x1sb = sbp.tile([P, F], fp32, tag="row")
                        nc.vector.tensor_copy(out=x1sb[:pr, :], in_=ps0[:pr, :])
                        nc.sync.dma_start(out=x1loc[t * P:t * P + pr, :],
                                          in_=x1sb[:pr, :])
                nc.sync.dma_start(out=x1loc[NSH:NSH + 1, :], in_=zrow[:])
                nc.gpsimd.collective_compute(
                    "AllGather", AL.bypass,
                    replica_groups=[list(range(NC_COUNT))],
                    ins=[x1loc.opt()], outs=[x1tab.opt()],
                )

            x1row0 = cst.tile([1, F], fp32)
            nc.sync.dma_start(out=x1row0[:], in_=x1tab[0:1, :])
            x1row0b = cst.tile([P, F], fp32)
            psb = psp.tile([P, F], fp32, tag="mm")
            nc.tensor.matmul(psb[:], ones1p[:], x1row0[:], start=True, stop=True)
            nc.vector.tensor_copy(out=x1row0b[:], in_=psb[:])

            def gather_reduce(idx_dram, t, table, zval, row0):
                """One 128-row tile: gather 32 rows/slot, tree-reduce, correct.
                Returns SBUF [P, F] f32 aggregate (uniform-softmax output)."""
                idx_t = idxp.tile([P, L], i32, tag="idx")
                nc.scalar.dma_start(out=idx_t[:], in_=idx_dram[t * P:(t + 1) * P, :])
                g = gbp.tile([P, L, F], fp32, tag="gb")
                for l in range(L):
                    nc.gpsimd.indirect_dma_start(
                        out=g[:, l, :],
                        out_offset=None,
                        in_=table[:],
                        in_offset=bass.IndirectOffsetOnAxis(
                            ap=idx_t[:, l:l + 1], axis=0),
                    )
                # in-place halving tree over the 32 slots
                h = L
                while h > 1:
                    h //= 2
                    nc.vector.tensor_tensor(
                        out=g[:, 0:h, :], in0=g[:, 0:h, :], in1=g[:, h:2 * h, :],
                        op=AL.add,
                    )
                # counts: slots remapped to zero row == nrows
                idx_f = stp.tile([P, L], fp32, tag="idxf")
                nc.vector.tensor_copy(out=idx_f[:], in_=idx_t[:])
                eqz = stp.tile([P, L], fp32, tag="eqz")
                nc.vector.tensor_scalar(
                    out=eqz[:], in0=idx_f[:], scalar1=float(zval), scalar2=None,
                    op0=AL.is_equal)
                n0 = stp.tile([P, 1], fp32, tag="n0")
                nc.vector.tensor_reduce(
                    out=n0[:], in_=eqz[:], axis=mybir.AxisListType.X, op=AL.add)
                cnt = stp.tile([P, 1], fp32, tag="cnt")
                nc.vector.tensor_scalar(
                    out=cnt[:], in0=n0[:], scalar1=-1.0, scalar2=float(L),
                    op0=AL.mult, op1=AL.add)
                cmax = stp.tile([P, 1], fp32, tag="cmax")
                nc.vector.tensor_scalar(
                    out=cmax[:], in0=cnt[:], scalar1=1.0, scalar2=None,
                    op0=AL.max)
                rec = stp.tile([P, 1], fp32, tag="rec")
                nc.vector.reciprocal(out=rec[:], in_=cmax[:])
                emp = stp.tile([P, 1], fp32, tag="emp")
                nc.vector.tensor_scalar(
                    out=emp[:], in0=cnt[:], scalar1=0.0, scalar2=None,
                    op0=AL.is_equal)
                agg = sbp.tile([P, F], fp32, tag="agg")
                nc.vector.tensor_scalar(
                    out=agg[:], in0=g[:, 0, :], scalar1=rec[:], scalar2=None,
                    op0=AL.mult)
                tmp = sbp.tile([P, F], fp32, tag="tmp")
                nc.vector.tensor_scalar(
                    out=tmp[:], in0=row0[:], scalar1=emp[:],
                    scalar2=None, op0=AL.mult)
                nc.vector.tensor_tensor(
                    out=agg[:], in0=agg[:], in1=tmp[:], op=AL.add)
                return agg

            # ---------- stage 1: edges ----------
            with nc.named_scope("stage1"):
                for t in range(N_TILES_EDGE):
                    agg = gather_reduce(seqp, t, x1tab, NSH, x1row0b)
                    edge = sbp.tile([P, F], fp32, tag="edge")
                    nc.vector.tensor_scalar(
                        out=edge[:], in0=agg[:], scalar1=0.0, scalar2=None,
                        op0=AL.max)  # relu
                    # e1 = edge @ W2 : transpose edge tile then matmul
                    ps2 = psp.tile([P, F], fp32, tag="mm")
                    for kc in range(2):
                        pst = pstp.tile([P, P], fp32, tag="tr")
                        nc.tensor.transpose(
                            out=pst[:], in_=edge[:, kc * P:(kc + 1) * P],
                            identity=ident[:])
                        edgeT = sbp.tile([P, P], fp32, tag="edgeT")
                        nc.vector.tensor_copy(out=edgeT[:], in_=pst[:])
                        nc.tensor.matmul(ps2[:], edgeT[:], w2sb[kc][:],
                                         start=(kc == 0), stop=(kc == 1))
                    pr = min(P, ESH - t * P)
                    if pr > 0:
                        e1sb = sbp.tile([P, F], fp32, tag="row")
                        nc.vector.tensor_copy(out=e1sb[:pr, :], in_=ps2[:pr, :])
                        nc.sync.dma_start(out=e1loc[t * P:t * P + pr, :],
                                          in_=e1sb[:pr, :])
                nc.sync.dma_start(out=e1loc[ESH:ESH + 1, :], in_=zrow[:])
                nc.gpsimd.collective_compute(
                    "AllGather", AL.bypass,
                    replica_groups=[list(range(NC_COUNT))],
                    ins=[e1loc.opt()], outs=[e1tab.opt()],
                )

            e1row0 = cst.tile([1, F], fp32)
            nc.sync.dma_start(out=e1row0[:], in_=e1tab[0:1, :])
            e1row0b = cst.tile([P, F], fp32)
            psb2 = psp.tile([P, F], fp32, tag="mm")
            nc.tensor.matmul(psb2[:], ones1p[:], e1row0[:], start=True, stop=True)
            nc.vector.tensor_copy(out=e1row0b[:], in_=psb2[:])

            # ---------- stage 2: nodes ----------
            with nc.named_scope("stage2"):
                for t in range(N_TILES_NODE):
                    agg = gather_reduce(useqp, t, e1tab, ESH, e1row0b)
                    nc.sync.dma_start(out=out[t * P:(t + 1) * P, :], in_=agg[:])

    nc.compile()
    return nc


def make_in_maps(x, seq, useq, W1, W2):
    x = np.asarray(x, dtype=np.float32)
    W1 = np.asarray(W1, dtype=np.float32)
    W2 = np.asarray(W2, dtype=np.float32)
    seq = np.asarray(seq)
    useq = np.asarray(useq)

    # map global ids into the AllGather table layout (shard + its zero row);
    # padding slots (id 0) -> the zero row at position NSH/ESH of shard 0
    seq_m = np.where(seq > 0,
                     (seq // NSH) * (NSH + 1) + seq % NSH,
                     NSH).astype(np.int32)
    useq_m = np.where(useq > 0,
                      (useq // ESH) * (ESH + 1) + useq % ESH,
                      ESH).astype(np.int32)

    xt = np.ascontiguousarray(x.T)  # [F, N]

    in_maps = []
    for c in range(NC_COUNT):
        xts = np.zeros((F, NSH_PAD), np.float32)
        xts[:, :NSH] = xt[:, c * NSH:(c + 1) * NSH]
        seqp = np.full((ESH_PAD, L), NSH, np.int32)
        seqp[:ESH] = seq_m[c * ESH:(c + 1) * ESH]
        useqp = np.full((NSH_PAD, L), ESH, np.int32)
        useqp[:NSH] = useq_m[c * NSH:(c + 1) * NSH]
        in_maps.append({
            "xts": xts,
            "w1": W1,
            "w2": W2,
            "seqp": seqp,
            "useqp": useqp,
        })
    return in_maps


def kernel(x, seq, useq, W1, W2):
    from concourse.bass_utils import run_bass_kernel_spmd

    in_maps = make_in_maps(x, seq, useq, W1, W2)
    nc = build_program()
    res = run_bass_kernel_spmd(nc, in_maps, core_ids=list(range(NC_COUNT)),
                               trace=False)
    parts = [res.results[c]["out"][:NSH] for c in range(NC_COUNT)]
    return np.concatenate(parts, axis=0)



# revision 6
# speedup vs baseline: 1.1698x; 1.1698x over previous
"""HGNN layer on 8 Trainium2 NeuronCores (Bass/Tile).

Reference computation:
    x1 = x @ W1                                    [N, F]
    w = softmax(where(seq > 0, 1, -9e15))          uniform over valid slots
    edge = relu(sum_l w[e,l] * x1[seq[e,l]])       [E, F]
    e1 = edge @ W2                                 [E, F]
    uw = softmax(where(useq > 0, 1, -9e15))
    node = sum_l uw[n,l] * e1[useq[n,l]]           [N, F]

Strategy (8-way SPMD), v4:
  - The slot-sum commutes with @W1, so gather raw x rows (no x1
    AllGather, no stage-0 matmul): edge = relu(((1/c)*sum_l x[seq]) @ W1).
  - Gathers are per-slot indirect DMAs ([128,1] offsets, one row per
    partition) -- measured to be the only offset form the SWDGE executes
    correctly; the kernel is bound by the ~1.1us Q7 descriptor-generation
    cost per gather, so everything else overlaps under the Pool engine.
  - Tables in bf16: 512B rows, half the HBM traffic of fp32.
  - Host precomputes uniform-softmax weights (1/count) and remaps padding
    slots to a zero table row; all index/weight tiles are bulk-loaded.
  - bf16 halving-tree reduce on DVE; scales/relu/PSUM copies on the
    scalar (ACT) engine; W1/W2 matmuls + transposes on PE in bf16.
  - e1 shard -> AllGather (bf16) -> stage-2 gather+reduce -> node out.
"""

import sys

sys.path.insert(0, "/opt/trn_rl_repo")

import numpy as np

N = 50000
E = 25000
F = 256
L = 32
P = 128
NC_COUNT = 8
NSH = N // NC_COUNT        # 6250 nodes per core
ESH = E // NC_COUNT        # 3125 edges per core
NSH_PAD = 6272             # 49 tiles
ESH_PAD = 3200             # 25 tiles
N_TILES_NODE = NSH_PAD // P
N_TILES_EDGE = ESH_PAD // P
XROWS = N + 1              # trailing zero row
EROWS = ESH + 1            # per-shard rows in the e1 table (incl zero row)


def build_program():
    from concourse import bacc, bass, mybir, tile  # noqa: F401
    from concourse.masks import make_identity

    fp32 = mybir.dt.float32
    bf16 = mybir.dt.bfloat16
    i32 = mybir.dt.int32
    AL = mybir.AluOpType
    AF = mybir.ActivationFunctionType

    nc = bacc.Bacc("TRN2", target_bir_lowering=False, debug=False,
                   num_devices=NC_COUNT)

    xtab = nc.dram_tensor("xtab", [XROWS, F], bf16, kind="ExternalInput").ap()
    w1 = nc.dram_tensor("w1", [F, F], bf16, kind="ExternalInput").ap()
    w2 = nc.dram_tensor("w2", [F, F], bf16, kind="ExternalInput").ap()
    sidx1 = nc.dram_tensor("sidx1", [P, N_TILES_EDGE, L], i32,
                           kind="ExternalInput").ap()
    sidx2 = nc.dram_tensor("sidx2", [P, N_TILES_NODE, L], i32,
                           kind="ExternalInput").ap()
    srec1 = nc.dram_tensor("srec1", [P, N_TILES_EDGE], fp32,
                           kind="ExternalInput").ap()
    srec2 = nc.dram_tensor("srec2", [P, N_TILES_NODE], fp32,
                           kind="ExternalInput").ap()
    out = nc.dram_tensor("out", [NSH_PAD, F], fp32, kind="ExternalOutput").ap()

    with tile.TileContext(nc) as tc:
        with (
            tc.tile_pool(name="cst", bufs=1) as cst,
            tc.tile_pool(name="gb", bufs=3) as gbp,
            tc.tile_pool(name="sb", bufs=3) as sbp,
            tc.tile_pool(name="ps", bufs=3, space="PSUM") as psp,
            tc.tile_pool(name="pst", bufs=2, space="PSUM") as pstp,
            tc.tile_pool(name="dram", bufs=1, space="DRAM") as dram,
        ):
            # ---------- constants ----------
            ident = cst.tile([P, P], bf16)
            make_identity(nc, ident[:])
            w1sb = [cst.tile([P, F], bf16, name=f"w1k{k}") for k in range(2)]
            w2sb = [cst.tile([P, F], bf16, name=f"w2k{k}") for k in range(2)]
            for k in range(2):
                nc.sync.dma_start(out=w1sb[k][:], in_=w1[k * P:(k + 1) * P, :])
                nc.sync.dma_start(out=w2sb[k][:], in_=w2[k * P:(k + 1) * P, :])
            zrow = cst.tile([1, F], bf16)
            nc.vector.memset(zrow[:], 0.0)

            # bulk-load all index / weight tiles
            i1 = cst.tile([P, N_TILES_EDGE, L], i32, name="i1")
            i2 = cst.tile([P, N_TILES_NODE, L], i32, name="i2")
            r1 = cst.tile([P, N_TILES_EDGE], fp32, name="r1")
            r2 = cst.tile([P, N_TILES_NODE], fp32, name="r2")
            nc.sync.dma_start(out=i1[:], in_=sidx1[:, :, :])
            nc.sync.dma_start(out=i2[:], in_=sidx2[:, :, :])
            nc.scalar.dma_start(out=r1[:], in_=srec1[:, :])
            nc.scalar.dma_start(out=r2[:], in_=srec2[:, :])

            # ---------- DRAM scratch ----------
            e1loc = dram.tile([EROWS, F], bf16)
            e1tab = dram.tile([NC_COUNT * EROWS, F], bf16,
                              addr_space="Shared")

            def gather_sum(idx_sb, t, table):
                """L per-slot indirect DMAs (one row per partition each,
                the only form the SWDGE executes correctly), then 5-level
                bf16 tree reduce. Sum lives in g[:, 0, :]."""
                g = gbp.tile([P, L, F], bf16, tag="g")
                for l in range(L):
                    nc.gpsimd.indirect_dma_start(
                        out=g[:, l, :],
                        out_offset=None,
                        in_=table[:],
                        in_offset=bass.IndirectOffsetOnAxis(
                            ap=idx_sb[:, t, l:l + 1], axis=0),
                    )
                with nc.allow_low_precision(
                        reason="bf16 tree reduce; tolerance 2e-2"):
                    h = L
                    while h > 1:
                        h //= 2
                        nc.vector.tensor_tensor(
                            out=g[:, 0:h, :], in0=g[:, 0:h, :],
                            in1=g[:, h:2 * h, :], op=AL.add,
                        )
                return g

            # ---------- stage 1: edges ----------
            with nc.named_scope("stage1"):
                for t in range(N_TILES_EDGE):
                    g = gather_sum(i1, t, xtab)
                    agg = sbp.tile([P, F], bf16, tag="agg")
                    nc.scalar.activation(out=agg[:], in_=g[:, 0, :],
                                         func=AF.Copy, scale=r1[:, t:t + 1])
                    # edge = relu(agg @ W1); e1 = edge @ W2
                    ps1 = psp.tile([P, F], fp32, tag="mm")
                    for kc in range(2):
                        pst = pstp.tile([P, P], bf16, tag="tr")
                        nc.tensor.transpose(
                            out=pst[:], in_=agg[:, kc * P:(kc + 1) * P],
                            identity=ident[:])
                        aggT = sbp.tile([P, P], bf16, tag="aggT")
                        nc.scalar.activation(out=aggT[:], in_=pst[:],
                                             func=AF.Copy)
                        nc.tensor.matmul(ps1[:], aggT[:], w1sb[kc][:],
                                         start=(kc == 0), stop=(kc == 1))
                    edge = sbp.tile([P, F], bf16, tag="edge")
                    nc.scalar.activation(out=edge[:], in_=ps1[:], func=AF.Relu)
                    ps2 = psp.tile([P, F], fp32, tag="mm")
                    for kc in range(2):
                        pst = pstp.tile([P, P], bf16, tag="tr")
                        nc.tensor.transpose(
                            out=pst[:], in_=edge[:, kc * P:(kc + 1) * P],
                            identity=ident[:])
                        edgeT = sbp.tile([P, P], bf16, tag="edgeT")
                        nc.scalar.activation(out=edgeT[:], in_=pst[:],
                                             func=AF.Copy)
                        nc.tensor.matmul(ps2[:], edgeT[:], w2sb[kc][:],
                                         start=(kc == 0), stop=(kc == 1))
                    pr = min(P, ESH - t * P)
                    if pr > 0:
                        e1sb = sbp.tile([P, F], bf16, tag="row")
                        nc.scalar.activation(out=e1sb[:], in_=ps2[:],
                                             func=AF.Copy)
                        nc.sync.dma_start(out=e1loc[t * P:t * P + pr, :],
                                          in_=e1sb[:pr, :])
                nc.sync.dma_start(out=e1loc[ESH:ESH + 1, :], in_=zrow[:])
                nc.gpsimd.collective_compute(
                    "AllGather", AL.bypass,
                    replica_groups=[list(range(NC_COUNT))],
                    ins=[e1loc.opt()], outs=[e1tab.opt()],
                )

            # ---------- stage 2: nodes ----------
            with nc.named_scope("stage2"):
                for t in range(N_TILES_NODE):
                    g = gather_sum(i2, t, e1tab)
                    node = sbp.tile([P, F], fp32, tag="node")
                    nc.scalar.activation(out=node[:], in_=g[:, 0, :],
                                         func=AF.Copy, scale=r2[:, t:t + 1])
                    nc.sync.dma_start(out=out[t * P:(t + 1) * P, :],
                                      in_=node[:])

    nc.compile()
    return nc


def _tile_idxs(M, n_tiles):
    """[n_tiles*P, L] int32 -> [P, n_tiles, L] (row t*P+p at [p, t, :])."""
    return np.ascontiguousarray(
        M.reshape(n_tiles, P, L).transpose(1, 0, 2).astype(np.int32))


def _tile_recs(R, n_tiles):
    """[n_tiles*P] f32 -> [P, n_tiles] f32 (row t*P+p at [p, t])."""
    return np.ascontiguousarray(R.reshape(n_tiles, P).T)


def make_in_maps(x, seq, useq, W1, W2):
    import ml_dtypes

    bf16 = ml_dtypes.bfloat16
    x = np.asarray(x, dtype=np.float32)
    W1b = np.asarray(W1, dtype=np.float32).astype(bf16)
    W2b = np.asarray(W2, dtype=np.float32).astype(bf16)
    seq = np.asarray(seq)
    useq = np.asarray(useq)

    xtab = np.zeros((XROWS, F), bf16)
    xtab[:N] = x.astype(bf16)

    # uniform softmax weights: 1/count over valid (id>0) slots; all-padding
    # rows keep idx 0 with weight 1/L (softmax degenerates to uniform and
    # every slot gathers row 0)
    cnt1 = (seq > 0).sum(axis=1)
    rec1 = np.where(cnt1 > 0, 1.0 / np.maximum(cnt1, 1), 1.0 / L)
    rec1 = rec1.astype(np.float32)
    seq_m = np.where(seq > 0, seq,
                     np.where(cnt1[:, None] > 0, N, 0)).astype(np.int32)

    # e1 table layout: 8 shards x EROWS rows, zero row at slot ESH of shard 0
    cnt2 = (useq > 0).sum(axis=1)
    rec2 = np.where(cnt2 > 0, 1.0 / np.maximum(cnt2, 1), 1.0 / L)
    rec2 = rec2.astype(np.float32)
    useq_m = np.where(useq > 0, (useq // ESH) * EROWS + useq % ESH,
                      np.where(cnt2[:, None] > 0, ESH, 0)).astype(np.int32)

    in_maps = []
    for c in range(NC_COUNT):
        s_c = np.full((ESH_PAD, L), N, np.int32)
        s_c[:ESH] = seq_m[c * ESH:(c + 1) * ESH]
        r1 = np.zeros(ESH_PAD, np.float32)
        r1[:ESH] = rec1[c * ESH:(c + 1) * ESH]
        u_c = np.full((NSH_PAD, L), ESH, np.int32)
        u_c[:NSH] = useq_m[c * NSH:(c + 1) * NSH]
        r2 = np.zeros(NSH_PAD, np.float32)
        r2[:NSH] = rec2[c * NSH:(c + 1) * NSH]
        in_maps.append({
            "xtab": xtab,
            "w1": W1b,
            "w2": W2b,
            "sidx1": _tile_idxs(s_c, N_TILES_EDGE),
            "sidx2": _tile_idxs(u_c, N_TILES_NODE),
            "srec1": _tile_recs(r1, N_TILES_EDGE),
            "srec2": _tile_recs(r2, N_TILES_NODE),
        })
    return in_maps


def kernel(x, seq, useq, W1, W2):
    from concourse.bass_utils import run_bass_kernel_spmd

    in_maps = make_in_maps(x, seq, useq, W1, W2)
    nc = build_program()
    res = run_bass_kernel_spmd(nc, in_maps, core_ids=list(range(NC_COUNT)),
                               trace=False)
    parts = [res.results[c]["out"][:NSH] for c in range(NC_COUNT)]
    return np.concatenate(parts, axis=0)


# revision 7
# speedup vs baseline: 1.1802x; 1.0089x over previous
"""HGNN layer on 8 Trainium2 NeuronCores (Bass/Tile).

Reference computation:
    x1 = x @ W1                                    [N, F]
    w = softmax(where(seq > 0, 1, -9e15))          uniform over valid slots
    edge = relu(sum_l w[e,l] * x1[seq[e,l]])       [E, F]
    e1 = edge @ W2                                 [E, F]
    uw = softmax(where(useq > 0, 1, -9e15))
    node = sum_l uw[n,l] * e1[useq[n,l]]           [N, F]

Strategy (8-way SPMD), v4:
  - The slot-sum commutes with @W1, so gather raw x rows (no x1
    AllGather, no stage-0 matmul): edge = relu(((1/c)*sum_l x[seq]) @ W1).
  - Gathers are per-slot indirect DMAs ([128,1] offsets, one row per
    partition) -- measured to be the only offset form the SWDGE executes
    correctly; the kernel is bound by the ~1.1us Q7 descriptor-generation
    cost per gather, so everything else overlaps under the Pool engine.
  - Tables in bf16: 512B rows, half the HBM traffic of fp32.
  - Host precomputes uniform-softmax weights (1/count) and remaps padding
    slots to a zero table row; all index/weight tiles are bulk-loaded.
  - bf16 halving-tree reduce on DVE; scales/relu/PSUM copies on the
    scalar (ACT) engine; W1/W2 matmuls + transposes on PE in bf16.
  - e1 shard -> AllGather (bf16) -> stage-2 gather+reduce -> node out.
"""

import sys

sys.path.insert(0, "/opt/trn_rl_repo")

import numpy as np

N = 50000
E = 25000
F = 256
L = 32
P = 128
NC_COUNT = 8
NSH = N // NC_COUNT        # 6250 nodes per core
ESH = E // NC_COUNT        # 3125 edges per core
NSH_PAD = 6272             # 49 tiles
ESH_PAD = 3200             # 25 tiles
N_TILES_NODE = NSH_PAD // P
N_TILES_EDGE = ESH_PAD // P
XROWS = N + 1              # trailing zero row
EROWS = ESH + 1            # per-shard rows in the e1 table (incl zero row)


def build_program():
    from concourse import bacc, bass, mybir, tile  # noqa: F401
    from concourse.masks import make_identity

    fp32 = mybir.dt.float32
    bf16 = mybir.dt.bfloat16
    i32 = mybir.dt.int32
    AL = mybir.AluOpType
    AF = mybir.ActivationFunctionType

    nc = bacc.Bacc("TRN2", target_bir_lowering=False, debug=False,
                   num_devices=NC_COUNT)

    xtab = nc.dram_tensor("xtab", [XROWS, F], bf16, kind="ExternalInput").ap()
    w1 = nc.dram_tensor("w1", [F, F], bf16, kind="ExternalInput").ap()
    w2 = nc.dram_tensor("w2", [F, F], bf16, kind="ExternalInput").ap()
    sidx1 = nc.dram_tensor("sidx1", [P, N_TILES_EDGE, L], i32,
                           kind="ExternalInput").ap()
    sidx2 = nc.dram_tensor("sidx2", [P, N_TILES_NODE, L], i32,
                           kind="ExternalInput").ap()
    srec1 = nc.dram_tensor("srec1", [P, N_TILES_EDGE], fp32,
                           kind="ExternalInput").ap()
    srec2 = nc.dram_tensor("srec2", [P, N_TILES_NODE], fp32,
                           kind="ExternalInput").ap()
    out = nc.dram_tensor("out", [NSH_PAD, F], fp32, kind="ExternalOutput").ap()

    with tile.TileContext(nc) as tc:
        with (
            tc.tile_pool(name="cst", bufs=1) as cst,
            tc.tile_pool(name="gb", bufs=4) as gbp,
            tc.tile_pool(name="sb", bufs=4) as sbp,
            tc.tile_pool(name="ps", bufs=4, space="PSUM") as psp,
            tc.tile_pool(name="pst", bufs=4, space="PSUM") as pstp,
            tc.tile_pool(name="dram", bufs=1, space="DRAM") as dram,
        ):
            # ---------- constants ----------
            ident = cst.tile([P, P], bf16)
            make_identity(nc, ident[:])
            w1sb = [cst.tile([P, F], bf16, name=f"w1k{k}") for k in range(2)]
            w2sb = [cst.tile([P, F], bf16, name=f"w2k{k}") for k in range(2)]
            for k in range(2):
                nc.sync.dma_start(out=w1sb[k][:], in_=w1[k * P:(k + 1) * P, :])
                nc.sync.dma_start(out=w2sb[k][:], in_=w2[k * P:(k + 1) * P, :])
            zrow = cst.tile([1, F], bf16)
            nc.vector.memset(zrow[:], 0.0)

            # bulk-load all index / weight tiles
            i1 = cst.tile([P, N_TILES_EDGE, L], i32, name="i1")
            i2 = cst.tile([P, N_TILES_NODE, L], i32, name="i2")
            r1 = cst.tile([P, N_TILES_EDGE], fp32, name="r1")
            r2 = cst.tile([P, N_TILES_NODE], fp32, name="r2")
            nc.sync.dma_start(out=i1[:], in_=sidx1[:, :, :])
            nc.sync.dma_start(out=i2[:], in_=sidx2[:, :, :])
            nc.scalar.dma_start(out=r1[:], in_=srec1[:, :])
            nc.scalar.dma_start(out=r2[:], in_=srec2[:, :])

            # ---------- DRAM scratch ----------
            e1loc = dram.tile([EROWS, F], bf16)
            e1tab = dram.tile([NC_COUNT * EROWS, F], bf16,
                              addr_space="Shared")

            def gather_sum(idx_sb, t, table):
                """L per-slot indirect DMAs (one row per partition each,
                the only form the SWDGE executes correctly), then 5-level
                bf16 tree reduce. Sum lives in g[:, 0, :]."""
                g = gbp.tile([P, L, F], bf16, tag="g")
                for l in range(L):
                    nc.gpsimd.indirect_dma_start(
                        out=g[:, l, :],
                        out_offset=None,
                        in_=table[:],
                        in_offset=bass.IndirectOffsetOnAxis(
                            ap=idx_sb[:, t, l:l + 1], axis=0),
                    )
                with nc.allow_low_precision(
                        reason="bf16 tree reduce; tolerance 2e-2"):
                    h = L
                    while h > 1:
                        h //= 2
                        nc.vector.tensor_tensor(
                            out=g[:, 0:h, :], in0=g[:, 0:h, :],
                            in1=g[:, h:2 * h, :], op=AL.add,
                        )
                return g

            # ---------- stage 1: edges ----------
            with nc.named_scope("stage1"):
                for t in range(N_TILES_EDGE):
                    g = gather_sum(i1, t, xtab)
                    agg = sbp.tile([P, F], bf16, tag="agg")
                    nc.scalar.activation(out=agg[:], in_=g[:, 0, :],
                                         func=AF.Copy, scale=r1[:, t:t + 1])
                    # edge = relu(agg @ W1); e1 = edge @ W2
                    ps1 = psp.tile([P, F], fp32, tag="mm")
                    for kc in range(2):
                        pst = pstp.tile([P, P], bf16, tag="tr")
                        nc.tensor.transpose(
                            out=pst[:], in_=agg[:, kc * P:(kc + 1) * P],
                            identity=ident[:])
                        aggT = sbp.tile([P, P], bf16, tag="aggT")
                        nc.scalar.activation(out=aggT[:], in_=pst[:],
                                             func=AF.Copy)
                        nc.tensor.matmul(ps1[:], aggT[:], w1sb[kc][:],
                                         start=(kc == 0), stop=(kc == 1))
                    edge = sbp.tile([P, F], bf16, tag="edge")
                    nc.scalar.activation(out=edge[:], in_=ps1[:], func=AF.Relu)
                    ps2 = psp.tile([P, F], fp32, tag="mm")
                    for kc in range(2):
                        pst = pstp.tile([P, P], bf16, tag="tr")
                        nc.tensor.transpose(
                            out=pst[:], in_=edge[:, kc * P:(kc + 1) * P],
                            identity=ident[:])
                        edgeT = sbp.tile([P, P], bf16, tag="edgeT")
                        nc.scalar.activation(out=edgeT[:], in_=pst[:],
                                             func=AF.Copy)
                        nc.tensor.matmul(ps2[:], edgeT[:], w2sb[kc][:],
                                         start=(kc == 0), stop=(kc == 1))
                    pr = min(P, ESH - t * P)
                    if pr > 0:
                        e1sb = sbp.tile([P, F], bf16, tag="row")
                        nc.scalar.activation(out=e1sb[:], in_=ps2[:],
                                             func=AF.Copy)
                        nc.sync.dma_start(out=e1loc[t * P:t * P + pr, :],
                                          in_=e1sb[:pr, :])
                nc.sync.dma_start(out=e1loc[ESH:ESH + 1, :], in_=zrow[:])
                nc.gpsimd.collective_compute(
                    "AllGather", AL.bypass,
                    replica_groups=[list(range(NC_COUNT))],
                    ins=[e1loc.opt()], outs=[e1tab.opt()],
                )

            # ---------- stage 2: nodes ----------
            with nc.named_scope("stage2"):
                for t in range(N_TILES_NODE):
                    g = gather_sum(i2, t, e1tab)
                    node = sbp.tile([P, F], fp32, tag="node")
                    nc.scalar.activation(out=node[:], in_=g[:, 0, :],
                                         func=AF.Copy, scale=r2[:, t:t + 1])
                    nc.sync.dma_start(out=out[t * P:(t + 1) * P, :],
                                      in_=node[:])

    nc.compile()
    return nc


def _tile_idxs(M, n_tiles):
    """[n_tiles*P, L] int32 -> [P, n_tiles, L] (row t*P+p at [p, t, :])."""
    return np.ascontiguousarray(
        M.reshape(n_tiles, P, L).transpose(1, 0, 2).astype(np.int32))


def _tile_recs(R, n_tiles):
    """[n_tiles*P] f32 -> [P, n_tiles] f32 (row t*P+p at [p, t])."""
    return np.ascontiguousarray(R.reshape(n_tiles, P).T)


def make_in_maps(x, seq, useq, W1, W2):
    import ml_dtypes

    bf16 = ml_dtypes.bfloat16
    x = np.asarray(x, dtype=np.float32)
    W1b = np.asarray(W1, dtype=np.float32).astype(bf16)
    W2b = np.asarray(W2, dtype=np.float32).astype(bf16)
    seq = np.asarray(seq)
    useq = np.asarray(useq)

    xtab = np.zeros((XROWS, F), bf16)
    xtab[:N] = x.astype(bf16)

    # uniform softmax weights: 1/count over valid (id>0) slots; all-padding
    # rows keep idx 0 with weight 1/L (softmax degenerates to uniform and
    # every slot gathers row 0)
    cnt1 = (seq > 0).sum(axis=1)
    rec1 = np.where(cnt1 > 0, 1.0 / np.maximum(cnt1, 1), 1.0 / L)
    rec1 = rec1.astype(np.float32)
    seq_m = np.where(seq > 0, seq,
                     np.where(cnt1[:, None] > 0, N, 0)).astype(np.int32)

    # e1 table layout: 8 shards x EROWS rows, zero row at slot ESH of shard 0
    cnt2 = (useq > 0).sum(axis=1)
    rec2 = np.where(cnt2 > 0, 1.0 / np.maximum(cnt2, 1), 1.0 / L)
    rec2 = rec2.astype(np.float32)
    useq_m = np.where(useq > 0, (useq // ESH) * EROWS + useq % ESH,
                      np.where(cnt2[:, None] > 0, ESH, 0)).astype(np.int32)

    in_maps = []
    for c in range(NC_COUNT):
        s_c = np.full((ESH_PAD, L), N, np.int32)
        s_c[:ESH] = seq_m[c * ESH:(c + 1) * ESH]
        r1 = np.zeros(ESH_PAD, np.float32)
        r1[:ESH] = rec1[c * ESH:(c + 1) * ESH]
        u_c = np.full((NSH_PAD, L), ESH, np.int32)
        u_c[:NSH] = useq_m[c * NSH:(c + 1) * NSH]
        r2 = np.zeros(NSH_PAD, np.float32)
        r2[:NSH] = rec2[c * NSH:(c + 1) * NSH]
        in_maps.append({
            "xtab": xtab,
            "w1": W1b,
            "w2": W2b,
            "sidx1": _tile_idxs(s_c, N_TILES_EDGE),
            "sidx2": _tile_idxs(u_c, N_TILES_NODE),
            "srec1": _tile_recs(r1, N_TILES_EDGE),
            "srec2": _tile_recs(r2, N_TILES_NODE),
        })
    return in_maps


def kernel(x, seq, useq, W1, W2):
    from concourse.bass_utils import run_bass_kernel_spmd

    in_maps = make_in_maps(x, seq, useq, W1, W2)
    nc = build_program()
    res = run_bass_kernel_spmd(nc, in_maps, core_ids=list(range(NC_COUNT)),
                               trace=False)
    parts = [res.results[c]["out"][:NSH] for c in range(NC_COUNT)]
    return np.concatenate(parts, axis=0)
